# revision 6
# baseline (speedup 1.0000x reference)
"""Trainium2 Bass/Tile kernel for the DAFMoE layer (dense all-expert FFN with
numeric/categorical preservation paths), data-parallel over the flattened token
dim across 8 NeuronCores.

Algorithm (per core, NLOC=2048 tokens):
  FFN path:  t1T[f,n] = sum_d w1[e,d,f] * hT[d,n]          (PE, fp16)
             u[f,n]   = gelu(t1T) * g[n,e]                  (ACT + DVE)
             accT[d,n]+= sum_{e,f} w2[e,f,d] * u_e[f,n]     (PE PSUM accum)
  Preservation paths are merged into ONE table gather:
             table[0:1000]    = tanh(c * w_num[e,d] + b_num[e,d])  (built on device)
             table[1000:2000] = omega_cat_emb transposed to [C, E*D]
             row(n) = table[r_n + 1000*(1-m_n)]
             accT[d,n] += sum_e (alpha_e*g[n,e]) * row(n)[e*D+d]
  implemented as per-expert diagonal matmuls accumulating into the same PSUM.
Output is produced transposed ([D, NLOC]) and re-transposed on host.
"""
import time

import numpy as np

import concourse.bass as bass
import concourse.tile as tile
from concourse import bacc, mybir

# ---- problem constants (hardcoded per contract) ----
B, S, D, E, DF, C = 8, 2048, 256, 8, 512, 1000
NCORES = 8
N = B * S
NLOC = N // NCORES      # 2048 tokens per core
NT = NLOC // 128        # 16 token tiles
NCH = NLOC // 512       # 4 n-chunks of 512
KT = D // 128           # 2 contraction tiles for stage A
FT = DF // 128          # 4 f tiles
DCH = D // 128          # 2 output-row chunks
ED = E * D              # 2048 = table row width
TROWS = 2 * C           # merged table rows

F16 = mybir.dt.float16
F32 = mybir.dt.float32
I32 = mybir.dt.int32
AF = mybir.ActivationFunctionType
ALU = mybir.AluOpType


def build_bass():
    """Build the per-core Bass program (SPMD: identical program, per-core data)."""
    nc = bacc.Bacc("TRN2", target_bir_lowering=False, debug=False,
                   num_devices=NCORES)

    # -------- DRAM I/O --------
    hT_d = nc.dram_tensor("hT", [D, NLOC], F16, kind="ExternalInput")
    w1_d = nc.dram_tensor("w1k", [KT, 128, E * DF], F16, kind="ExternalInput")
    w2_d = nc.dram_tensor("w2f", [FT, 128, ED], F16, kind="ExternalInput")
    gT_d = nc.dram_tensor("gT", [E, NLOC], F16, kind="ExternalInput")
    gsh_d = nc.dram_tensor("gsh", [128, NT * E], F32, kind="ExternalInput")
    rm_d = nc.dram_tensor("rm", [128, 2 * NT], F32, kind="ExternalInput")
    prm_d = nc.dram_tensor("prm", [1, 4 * E], F32, kind="ExternalInput")
    nb_d = nc.dram_tensor("nb", [2, ED], F32, kind="ExternalInput")
    emb_d = nc.dram_tensor("embT", [C, ED], F16, kind="ExternalInput")
    on16_d = nc.dram_tensor("ones16", [1, 128], F16, kind="ExternalInput")
    on32_d = nc.dram_tensor("ones32", [1, 128], F32, kind="ExternalInput")
    eye_d = nc.dram_tensor("eye16", [128, 128], F16, kind="ExternalInput")
    outT_d = nc.dram_tensor("outT", [D, NLOC], F32, kind="ExternalOutput")
    table_d = nc.dram_tensor("table", [TROWS, ED], F16)  # internal scratch

    with tile.TileContext(nc) as tc:
        with tc.tile_pool(name="pers", bufs=1) as pers:
            # ---- persistent SBUF tensors ----
            w1s = [pers.tile([128, E * DF], F16, tag=f"w1s{k}", name=f"w1s{k}") for k in range(KT)]
            w2s = [pers.tile([128, ED], F16, tag=f"w2s{f}", name=f"w2s{f}") for f in range(FT)]
            hTs = [pers.tile([128, NLOC], F16, tag=f"hTs{k}", name=f"hTs{k}") for k in range(KT)]
            gTr = [pers.tile([1, NLOC], F16, tag=f"gTr{e}", name=f"gTr{e}")
                   for e in range(E)]
            gsh = pers.tile([128, NT * E], F32, tag="gsh", name="gsh")
            rms = pers.tile([128, 2 * NT], F32, tag="rms", name="rms")
            prs = pers.tile([1, 4 * E], F32, tag="prs", name="prs")
            nbw = pers.tile([1, ED], F32, tag="nbw", name="nbw")
            nbb = pers.tile([1, ED], F32, tag="nbb", name="nbb")
            on16 = pers.tile([1, 128], F16, tag="on16", name="on16")
            on32 = pers.tile([1, 128], F32, tag="on32", name="on32")
            eye = pers.tile([128, 128], F16, tag="eye", name="eye")
            Gb = pers.tile([128, E * NLOC], F16, tag="Gb", name="Gb")
            WB = pers.tile([128, ED], F32, tag="WB", name="WB")
            BB = pers.tile([128, ED], F32, tag="BB", name="BB")
            walpha = pers.tile([128, NT * E], F32, tag="walpha", name="walpha")
            idx = pers.tile([128, NT], I32, tag="idx", name="idx")
            alphab = pers.tile([128, E], F32, tag="alphab", name="alphab")

            for k in range(KT):
                nc.sync.dma_start(w1s[k][:], w1_d[k])
                nc.sync.dma_start(hTs[k][:], hT_d[k * 128:(k + 1) * 128, :])
            for f in range(FT):
                nc.sync.dma_start(w2s[f][:], w2_d[f])
            for e in range(E):
                nc.sync.dma_start(gTr[e][:], gT_d[e:e + 1, :])
            nc.sync.dma_start(gsh[:], gsh_d[:, :])
            nc.sync.dma_start(rms[:], rm_d[:, :])
            nc.sync.dma_start(prs[:], prm_d[:, :])
            nc.sync.dma_start(nbw[:], nb_d[0:1, :])
            nc.sync.dma_start(nbb[:], nb_d[1:2, :])
            nc.sync.dma_start(on16[:], on16_d[:, :])
            nc.sync.dma_start(on32[:], on32_d[:, :])
            nc.sync.dma_start(eye[:], eye_d[:, :])

            # ================= SETUP =================
            with tc.tile_pool(name="setup", bufs=2) as setup, \
                 tc.tile_pool(name="psetup", bufs=2, space="PSUM") as psetup, \
                 tc.tile_pool(name="tbuild", bufs=2) as tbuild:

                # alpha_e = sigmoid(steep * (|sigmoid(mu)-0.5| - thr))  [1,E]
                sg = setup.tile([1, E], F32, tag="sg", name="sg")
                nc.scalar.activation(sg[:], prs[0:1, 0:E], AF.Sigmoid)
                dist = setup.tile([1, E], F32, tag="dist", name="dist")
                nc.vector.tensor_scalar(dist[:], sg[:], -0.5, None, ALU.add)
                nc.scalar.activation(dist[:], dist[:], AF.Abs)
                targ0 = setup.tile([1, E], F32, tag="targ0", name="targ0")
                nc.vector.tensor_sub(targ0[:], dist[:], prs[0:1, 2 * E:3 * E])
                nc.vector.tensor_mul(targ0[:], targ0[:], prs[0:1, E:2 * E])
                alpha = setup.tile([1, E], F32, tag="alpha", name="alpha")
                nc.scalar.activation(alpha[:], targ0[:], AF.Sigmoid)

                # broadcast alpha across 128 partitions via PE outer product
                psa = psetup.tile([128, E], F32, tag="ps_small", name="ps_small")
                nc.tensor.matmul(psa[:], lhsT=on32[:], rhs=alpha[:],
                                 start=True, stop=True)
                nc.vector.tensor_copy(alphab[:], psa[:])

                # walpha[:, nt*E+e] = g[nt*128+p, e] * alpha_e   (fp16)
                for nt in range(NT):
                    nc.vector.tensor_mul(walpha[:, nt * E:(nt + 1) * E],
                                         gsh[:, nt * E:(nt + 1) * E], alphab[:])

                # merged gather index: idx = r + 1000 - 1000*m
                idxf = setup.tile([128, NT], F32, tag="idxf", name="idxf")
                nc.vector.tensor_scalar(idxf[:], rms[:, NT:2 * NT],
                                        -1000.0, 1000.0, ALU.mult, ALU.add)
                nc.vector.tensor_add(idxf[:], idxf[:], rms[:, 0:NT])
                nc.vector.tensor_copy(idx[:], idxf[:])

                # WB/BB: broadcast omega_num w/b rows across partitions
                for src_row, dest in ((nbw, WB), (nbb, BB)):
                    for ch in range(ED // 512):
                        psb = psetup.tile([128, 512], F32, tag="ps_big", name="ps_big")
                        nc.tensor.matmul(
                            psb[:], lhsT=on32[:],
                            rhs=src_row[0:1, ch * 512:(ch + 1) * 512],
                            start=True, stop=True)
                        nc.vector.tensor_copy(dest[:, ch * 512:(ch + 1) * 512],
                                              psb[:])

                # Gb[:, e*NLOC + n] = g[n, e] broadcast across partitions
                for e in range(E):
                    for ch in range(NLOC // 512):
                        psb = psetup.tile([128, 512], F32, tag="ps_big", name="ps_big")
                        nc.tensor.matmul(
                            psb[:], lhsT=on16[:],
                            rhs=gTr[e][0:1, ch * 512:(ch + 1) * 512],
                            start=True, stop=True)
                        nc.scalar.copy(
                            Gb[:, e * NLOC + ch * 512:e * NLOC + (ch + 1) * 512],
                            psb[:])

                # ---- build merged table ----
                # rows 0..999: tanh(c * wnum + bnum); rows 1000..1999: embT
                for ct in range((C + 127) // 128):
                    rows = min(128, C - ct * 128)
                    cci = setup.tile([128, 1], I32, tag="cci", name="cci")
                    nc.gpsimd.iota(cci[:], pattern=[[0, 1]], base=ct * 128,
                                   channel_multiplier=1)
                    ccf = setup.tile([128, 1], F32, tag="ccf", name="ccf")
                    nc.vector.tensor_copy(ccf[:], cci[:])
                    ta = tbuild.tile([128, ED], F32, tag="ta", name="ta")
                    nc.vector.tensor_scalar(ta[:rows], WB[:rows],
                                            ccf[:rows, 0:1], None, ALU.mult)
                    nc.vector.tensor_add(ta[:rows], ta[:rows], BB[:rows])
                    t16 = tbuild.tile([128, ED], F16, tag="t16", name="t16")
                    nc.scalar.activation(t16[:rows], ta[:rows], AF.Tanh)
                    nc.scalar.dma_start(table_d[ct * 128:ct * 128 + rows, :],
                                        t16[:rows])
                    # bounce-copy the embedding half of the table
                    bt = tbuild.tile([128, ED], F16, tag="bt", name="bt")
                    nc.sync.dma_start(bt[:rows], emb_d[ct * 128:ct * 128 + rows, :])
                    nc.scalar.dma_start(
                        table_d[C + ct * 128:C + ct * 128 + rows, :], bt[:rows])

            # ================= MAIN =================
            with tc.tile_pool(name="psA", bufs=6, space="PSUM") as psA, \
                 tc.tile_pool(name="accp", bufs=2, space="PSUM") as accp, \
                 tc.tile_pool(name="upool", bufs=6) as upool, \
                 tc.tile_pool(name="ugpool", bufs=6) as ugpool, \
                 tc.tile_pool(name="gpool", bufs=3) as gpool, \
                 tc.tile_pool(name="dpool", bufs=4) as dpool, \
                 tc.tile_pool(name="opool", bufs=4) as opool:

                for nch in range(NCH):
                    ncol = slice(nch * 512, (nch + 1) * 512)
                    accs = [accp.tile([128, 512], F32, tag="acc", name="acc")
                            for _ in range(DCH)]
                    for e in range(E):
                        pas = [psA.tile([128, 512], F32, tag="psA", name="psA")
                               for _ in range(FT)]
                        for ft in range(FT):
                            for kt in range(KT):
                                nc.tensor.matmul(
                                    pas[ft][:],
                                    lhsT=w1s[kt][:, e * DF + ft * 128:
                                                 e * DF + (ft + 1) * 128],
                                    rhs=hTs[kt][:, ncol],
                                    start=(kt == 0), stop=(kt == KT - 1))
                        us = []
                        for ft in range(FT):
                            ug = ugpool.tile([128, 512], F16, tag="ug", name="ug")
                            nc.scalar.activation(ug[:], pas[ft][:], AF.Gelu)
                            u = upool.tile([128, 512], F16, tag="u", name="u")
                            nc.vector.tensor_mul(
                                u[:], ug[:],
                                Gb[:, e * NLOC + nch * 512:
                                   e * NLOC + (nch + 1) * 512])
                            us.append(u)
                        for dch in range(DCH):
                            for ft in range(FT):
                                nc.tensor.matmul(
                                    accs[dch][:],
                                    lhsT=w2s[ft][:, e * D + dch * 128:
                                                 e * D + (dch + 1) * 128],
                                    rhs=us[ft][:],
                                    start=(e == 0 and ft == 0), stop=False,
                                    skip_group_check=True)

                    # preservation path: gather + diagonal matmuls
                    for ntl in range(4):
                        nt = nch * 4 + ntl
                        gt = gpool.tile([128, ED], F16, tag="gt", name="gt")
                        nc.gpsimd.indirect_dma_start(
                            out=gt[:], out_offset=None, in_=table_d[:, :],
                            in_offset=bass.IndirectOffsetOnAxis(
                                ap=idx[:, nt:nt + 1], axis=0))
                        for e in range(E):
                            dg = dpool.tile([128, 128], F16, tag="dg", name="dg")
                            nc.vector.tensor_scalar(
                                dg[:], eye[:], walpha[:, nt * E + e:nt * E + e + 1],
                                None, ALU.mult)
                            for dch in range(DCH):
                                nc.tensor.matmul(
                                    accs[dch][:, ntl * 128:(ntl + 1) * 128],
                                    lhsT=gt[:, e * D + dch * 128:
                                            e * D + (dch + 1) * 128],
                                    rhs=dg[:],
                                    start=False, stop=(e == E - 1),
                                    skip_group_check=True)

                    for dch in range(DCH):
                        ot = opool.tile([128, 512], F32, tag="ot", name="ot")
                        nc.scalar.copy(ot[:], accs[dch][:])
                        nc.scalar.dma_start(
                            outT_d[dch * 128:(dch + 1) * 128, ncol], ot[:])

    nc.compile()
    return nc


_NC_CACHE = None


def _get_nc():
    global _NC_CACHE
    if _NC_CACHE is None:
        _NC_CACHE = build_bass()
    return _NC_CACHE


def stage_inputs(inputs):
    """Host-side layout staging: shard + transpose + cast. Returns in_maps."""
    h = np.asarray(inputs["h"], np.float32)
    g = np.asarray(inputs["gating_weights"], np.float32)
    mu = np.asarray(inputs["mu"], np.float32)
    r_j = np.asarray(inputs["r_j"], np.float32)
    fmask = np.asarray(inputs["feature_mask"], np.float32)
    w1 = np.asarray(inputs["w1"], np.float32)
    w2 = np.asarray(inputs["w2"], np.float32)
    onw = np.asarray(inputs["omega_num_w"], np.float32)
    onb = np.asarray(inputs["omega_num_b"], np.float32)
    emb = np.asarray(inputs["omega_cat_emb"], np.float32)
    gs = np.asarray(inputs["gate_steepness"], np.float32)
    gt = np.asarray(inputs["gate_threshold"], np.float32)

    hf = h.reshape(N, D)
    gf = g.reshape(N, E)
    rf = r_j.reshape(N)
    mf = fmask.reshape(N)

    # replicated tensors
    w1k = np.ascontiguousarray(
        w1.transpose(1, 0, 2).reshape(D, E * DF)).astype(np.float16).reshape(
        KT, 128, E * DF)
    w2f = np.ascontiguousarray(
        w2.transpose(1, 0, 2).reshape(DF, ED)).astype(np.float16).reshape(
        FT, 128, ED)
    prm = np.zeros((1, 4 * E), np.float32)
    prm[0, 0:E], prm[0, E:2 * E], prm[0, 2 * E:3 * E] = mu, gs, gt
    nb = np.stack([onw.reshape(ED), onb.reshape(ED)]).astype(np.float32)
    embT = np.ascontiguousarray(
        emb.transpose(1, 0, 2).reshape(C, ED)).astype(np.float16)
    on16 = np.ones((1, 128), np.float16)
    on32 = np.ones((1, 128), np.float32)
    eye16 = np.eye(128, dtype=np.float16)

    in_maps = []
    for i in range(NCORES):
        sl = slice(i * NLOC, (i + 1) * NLOC)
        hT = np.ascontiguousarray(hf[sl].T).astype(np.float16)
        gloc = gf[sl]
        gT = np.ascontiguousarray(gloc.T).astype(np.float16)
        gsh = np.ascontiguousarray(
            gloc.reshape(NT, 128, E).transpose(1, 0, 2).reshape(128, NT * E))
        rm = np.concatenate([rf[sl].reshape(NT, 128).T,
                             mf[sl].reshape(NT, 128).T], axis=1)
        rm = np.ascontiguousarray(rm).astype(np.float32)
        in_maps.append(dict(
            hT=hT, w1k=w1k, w2f=w2f, gT=gT, gsh=gsh, rm=rm, prm=prm, nb=nb,
            embT=embT, ones16=on16, ones32=on32, eye16=eye16))
    return in_maps


def assemble(results):
    out = np.empty((N, D), np.float32)
    for i in range(NCORES):
        out[i * NLOC:(i + 1) * NLOC] = results[i]["outT"].T
    return out.reshape(B, S, D)


def kernel(**inputs):
    from concourse.bass_utils import run_bass_kernel_spmd
    nc = _get_nc()
    in_maps = stage_inputs(inputs)
    res = run_bass_kernel_spmd(nc, in_maps, list(range(NCORES)))
    return assemble(res.results)
